# revision 57
# baseline (speedup 1.0000x reference)
"""AdderConv (AdderNet conv 3x3 + BatchNorm2d, training stats) on 8 trn2 cores.

Reference:
  u[n,o,yx] = sum_{c,dy,dx} |x[n,c,y+dy-1,x+dx-1] - W[o,c,dy,dx]|   (zero pad)
  out = -u, then BatchNorm2d over (n,y,x) per channel o with affine gamma/beta.

Sharding: output channels. Core k owns channels [8k, 8k+8); every core reads the
full x. BatchNorm stats are per-channel, hence fully core-local.

Structure (tuned for the Bass cost model; 60.5us baseline -> ~28.6us):
  - TRANSPOSED matmuls: cost-model matmul time = OUT free size x pe_cycle
    (independent of contraction K) and Ldweights is free. The reduction over
    the 128 (slot,channel) partitions therefore runs with the production tile
    as the STATIONARY operand (chunked [128,112]) and a tiny [128,8] +/-2
    slot-selection matrix as the MOVING operand: 8 cycles per matmul instead
    of 392. PSUM holds u TRANSPOSED: [112 spatial, 8img, 7chunk, 8ch] f32 =
    1792B, a single bank for the whole batch. PE drops from ~50us (baseline,
    where it was the bottleneck) to ~7us; elementwise production becomes the
    bottleneck and is split across all three vector engines.
  - production, one [128, n_img*28*28] bf16 op per (group,tap) unit:
      DVE/Pool taps: min(x,w) tensor_scalar; |x-w| = x + w - 2min: the matmul
        applies -2 via the selection matrix, the w-sum is image-independent
        per channel (host compensates any per-image engine splits through sx)
        and is absorbed by BN, and the x-sum S_x is accumulated into psum by
        free [112,112]-identity x [112,8]-sx matmuls from a host-built f32
        tensor replicated per output channel.
      ACT taps: |x + (-w)| via activation(Abs, bias), selection matrix +1.
    DVE runs 11 of the 12 dx!=1 taps in the 4x bf16 mode (0.26 ns/col);
    ACT/Pool (Pool tensor_scalar prices at efficiency 1.0) split the rest.
    Each unit op splits into an imgs-0:2 stage (starts as soon as the first
    xx DMA lands) and imgs-2:8 stages; per-engine tile pools (bufs=4) keep
    the engines from serializing on buffer rotation.
  - psum protocol: explicit head memsets + all matmuls start=False with
    skip_group_check: a first write to a virgin element either accumulates
    onto the memset zero (stale has_written=1) or overwrites (=0) - correct
    under either hardware semantic, and group-free for the simulator (which
    otherwise forbids mid-group evacuation reads and interleaved groups in
    one 2KB zero region).
  - BN stats via matmuls after a 2-image psum->SBUF evacuation (DVE/ACT
    alternating): per (img,chunk) lhsT = u-chunk [112,8]; rhs = u-chunk
    -> S2 += u u^T (diag = sum u^2), rhs = ones -> S1 += sum u.
    var = diag(S2)/N - mean^2 via identity-mask multiply + row reduce.
  - affine on the transposed layout: A = -gamma*rstd, B = beta - A*mean
    become diag matrices via one tensor_scalar (A x identity) each, then
    broadcast to [112, 7, 8] by K=8 all-ones matmuls (no transpose step);
    y = u*A_b + B_b is two tensor_tensor ops per image, all on Pool in
    image order (serial single-engine beats cross-engine splits here); the
    imgs-0:4 output DMA issues mid-stream on the ACT queue and imgs-4:8
    last on the SP queue.
"""

import os
import sys

import numpy as np

for _p in ("/opt/trn_rl_repo",):
    if os.path.isdir(_p) and _p not in sys.path:
        sys.path.insert(0, _p)

import concourse.bacc as bacc
import concourse.bass as bass
import concourse.tile as tile
from concourse import mybir
from concourse.bass_utils import run_bass_kernel_spmd

F32 = mybir.dt.float32
BF16 = mybir.dt.bfloat16
ALU = mybir.AluOpType
ACTF = mybir.ActivationFunctionType

N_CORES = 8
N_IMG = 8
C_IN = 32
O_TOT = 64
O_PER_CORE = O_TOT // N_CORES  # 8
N_GRP = 2                      # 2 groups of 4 channels (128 = 4*32 partitions)
HW = 28
S = HW * HW                    # 784
CK = 112                       # psum chunk width; 7 chunks of 112 per image
NCK = S // CK                  # 7
HP, WP = HW + 2, 32            # padded image rows=30, row stride 32
PADN = HP * WP                 # 960
NTOT = float(N_IMG * S)        # BN sample count per channel
EPS = 1e-5

STAGE_SPLIT = 2                # stage A = imgs [0,2), stage B = imgs [2,8)

# f32 param blob column layout
PF_COLS = 160
PF_WT = 0        # [128, 18] w  (unit u = g*9+j at col u)
PF_NWT = 18      # [128, 18] -w (ACT Abs bias)
PF_NGAM = 36     # [8, 1] -gamma
PF_BETA = 37     # [8, 1] beta
PF_EPS = 38      # [8, 1] eps
PF_ONE = 39      # [128, 1] ones (stats rhs)
PF_ONEROW = 40   # [1, 112] ones on partition 0 (broadcast lhsT)
PF_I8 = 152      # [8, 8] identity
# bf16 param blob column layout: selection matrices
PB_COLS = 32
PB_M2G = 0       # [128, 8] -2*G per group at 8g (min units)
PB_G = 16        # [128, 8] +1*G per group at 16+8g (abs units)


def _op_list():
    """Production ops in emission order.

    Returns (ops_a, ops_b) where each op = (engine, g, j, img_lo, img_hi) and
    engine in {'V' (DVE, min), 'A' (ACT, abs), 'P' (Pool, min)}. Pool's
    tensor_scalar prices at efficiency 1.0 (same rate as ACT, cheaper init),
    so it gets a dx!=1 unit too. Stage-B emission interleaves engines so the
    PE consumes each engine's tiles close to production order.
    """
    dve = [(g, j) for g in range(N_GRP) for j in range(9)
           if j % 3 != 1 and (g, j) != (1, 0)]                     # 11 units
    act = [(0, 1), (0, 4), (0, 7)]
    pool = [(1, 4), (1, 7), (1, 0)]
    ops_a, ops_b = [], []
    for g, j in dve:
        ops_a.append(("V", g, j, 0, STAGE_SPLIT))
    for g, j in act:
        ops_a.append(("A", g, j, 0, STAGE_SPLIT))
    for g, j in pool:
        ops_a.append(("P", g, j, 0, STAGE_SPLIT))
    ops_a.append(("A", 1, 1, 0, STAGE_SPLIT))
    # stage B, interleaved ~3 DVE ops per (ACT, Pool) pair. Each engine's
    # final op is split so early images complete (and evacuate) sooner.
    vb = [("V", g, j, STAGE_SPLIT, N_IMG) for g, j in dve[:-1]]
    vb += [("V", dve[-1][0], dve[-1][1], 2, 5), ("V", dve[-1][0], dve[-1][1], 5, 8)]
    ab = [("A", g, j, STAGE_SPLIT, N_IMG) for g, j in act[:-1]]
    ab += [("A", act[-1][0], act[-1][1], 2, 5), ("A", act[-1][0], act[-1][1], 5, 8)]
    pb_ = [("P", g, j, STAGE_SPLIT, N_IMG) for g, j in pool[:-1]]
    pb_ += [("P", pool[-1][0], pool[-1][1], 2, 5),
            ("P", pool[-1][0], pool[-1][1], 5, 8), ("P", 1, 1, 2, 5),
            ("P", 1, 1, 5, 8)]
    while vb or ab or pb_:
        for _ in range(3):
            if vb:
                ops_b.append(vb.pop(0))
        if ab:
            ops_b.append(ab.pop(0))
        if pb_:
            ops_b.append(pb_.pop(0))
    return ops_a, ops_b


def _min_taps(g, img):
    """Taps of group g computed with the min trick for image img."""
    taps = set()
    for ops in _op_list():
        for eng, gg, j, i0, i1 in ops:
            if gg == g and i0 <= img < i1 and eng in ("V", "P"):
                taps.add(j)
    return taps


def _build_nc() -> bass.Bass:
    nc = bacc.Bacc()
    xx_in = nc.declare_dram_parameter("xx", [128, N_IMG * PADN], BF16, isOutput=False)
    sx_in = nc.declare_dram_parameter(
        "sx", [CK, N_IMG, NCK, O_PER_CORE], F32, isOutput=False
    )
    id_in = nc.declare_dram_parameter("idm", [CK, CK], F32, isOutput=False)
    pf_in = nc.declare_dram_parameter("pf", [128, PF_COLS], F32, isOutput=False)
    pb_in = nc.declare_dram_parameter("pb", [128, PB_COLS], BF16, isOutput=False)
    y_out = nc.declare_dram_parameter(
        "y", [CK, N_IMG, NCK, O_PER_CORE], BF16, isOutput=True
    )

    ops_a, ops_b = _op_list()

    with tile.TileContext(nc) as tc:
        with (
            tc.tile_pool(name="singles", bufs=1) as singles,
            tc.tile_pool(name="dpv", bufs=6) as dp_v,
            tc.tile_pool(name="dpa", bufs=6) as dp_a,
            tc.tile_pool(name="dpp", bufs=6) as dp_p,
            tc.tile_pool(name="tpool", bufs=3) as t_pool,
            tc.tile_pool(name="ups", bufs=1, space="PSUM") as ups_pool,
            tc.tile_pool(name="sps", bufs=1, space="PSUM") as sps_pool,
            tc.tile_pool(name="bps", bufs=1, space="PSUM") as bps_pool,
            tc.tile_pool(name="small", bufs=1) as small,
        ):
            # --- head DMAs -------------------------------------------------
            pf = singles.tile([128, PF_COLS], F32)
            pb = singles.tile([128, PB_COLS], BF16)
            sx = singles.tile([CK, N_IMG, NCK, O_PER_CORE], F32)
            xxt = singles.tile([128, N_IMG, HP, WP], BF16)
            xf = xxt.rearrange("p a b c -> p (a b c)")
            # imgs 0-1 first so stage-A production starts ASAP; params on
            # the ACT queue ahead of the table preload
            nc.sync.dma_start(
                out=xf[:, : STAGE_SPLIT * PADN], in_=xx_in[:, : STAGE_SPLIT * PADN]
            )
            nc.scalar.dma_start(out=pf, in_=pf_in[:])
            nc.scalar.dma_start(out=pb, in_=pb_in[:])
            nc.sync.dma_start(
                out=xf[:, STAGE_SPLIT * PADN : 5 * PADN],
                in_=xx_in[:, STAGE_SPLIT * PADN : 5 * PADN],
            )
            nc.sync.dma_start(out=xf[:, 5 * PADN :], in_=xx_in[:, 5 * PADN :])
            idm = singles.tile([CK, CK], F32)
            nc.gpsimd.dma_start(out=sx, in_=sx_in[:])
            nc.gpsimd.dma_start(out=idm, in_=id_in[:])

            wt = pf[:, PF_WT : PF_WT + 18]
            nwt = pf[:, PF_NWT : PF_NWT + 18]
            ngam = pf[0:O_PER_CORE, PF_NGAM : PF_NGAM + 1]
            beta = pf[0:O_PER_CORE, PF_BETA : PF_BETA + 1]
            eps_sb = pf[0:O_PER_CORE, PF_EPS : PF_EPS + 1]
            ones_col = pf[0:CK, PF_ONE : PF_ONE + 1]
            ones8r = pf[0:O_PER_CORE, PF_ONEROW : PF_ONEROW + CK]
            i8 = pf[0:O_PER_CORE, PF_I8 : PF_I8 + 8]
            m2g = [pb[:, PB_M2G + 8 * g : PB_M2G + 8 * g + 8] for g in range(N_GRP)]
            gsel = [pb[:, PB_G + 8 * g : PB_G + 8 * g + 8] for g in range(N_GRP)]

            u_pair = [
                singles.tile([CK, 2, NCK, O_PER_CORE], F32, name=f"u{i}")
                for i in range(N_IMG // 2)
            ]
            y_sb = singles.tile([CK, N_IMG, NCK, O_PER_CORE], BF16)
            ab_sb = small.tile([O_PER_CORE, 2], F32)
            ab_bc = small.tile([CK, 2, NCK, O_PER_CORE], F32)

            # ACT table preload during DMA dead time (Sqrt/Abs/Copy/Identity
            # in one set -> no mid-kernel table swaps).
            tjunk = small.tile([8, 1], F32)
            nc.vector.memset(tjunk, 1.0)
            nc.scalar.activation(out=tjunk, in_=tjunk, func=ACTF.Sqrt, scale=1.0)
            nc.scalar.activation(out=tjunk, in_=tjunk, func=ACTF.Abs, scale=1.0)

            # PSUM tiles, each its own bank. All matmuls run start=False with
            # an explicit head memset: a first write to a virgin element
            # either accumulates onto the memset zero (stale has_written=1)
            # or overwrites (has_written=0) - correct under either hardware
            # semantic, and group-free for the simulator.
            u_ps_raw0 = ups_pool.tile([128, 512], F32)
            u_ps_raw1 = ups_pool.tile([128, 512], F32)
            _half = N_IMG // 2
            u_ps_banks = [
                r[0:CK, 0 : _half * NCK * O_PER_CORE].rearrange(
                    "p (i c o) -> p i c o", i=_half, c=NCK
                )
                for r in (u_ps_raw0, u_ps_raw1)
            ]

            def u_ps_at(img):
                return u_ps_banks[img // _half], img % _half

            s_ps_raw = sps_pool.tile([128, 512], F32)
            s2_ps = s_ps_raw[0:O_PER_CORE, 0:8]   # S2 = sum u u^T
            s1_ps = s_ps_raw[0:O_PER_CORE, 8:9]   # S1 = sum u
            ab_ps = s_ps_raw[0:1, 16:32]          # A,B rows (bank reused post-stats)
            b_ps_raw = bps_pool.tile([128, 512], F32)
            abc_ps = b_ps_raw[0:CK, 0 : 2 * NCK * O_PER_CORE].rearrange(
                "p (t c o) -> p t c o", t=2, c=NCK
            )
            nc.vector.memset(u_ps_raw0, 0.0)
            nc.vector.memset(u_ps_raw1, 0.0)
            nc.vector.memset(s_ps_raw, 0.0)
            nc.vector.memset(b_ps_raw, 0.0)

            # --- production + reduction -----------------------------------

            def emit_unit(eng, g, j, i0, i1):
                u = g * 9 + j
                dy, dx = divmod(j, 3)
                ni = i1 - i0
                win = xxt[:, i0:i1, dy : dy + HW, dx : dx + HW]
                d_t = {"V": dp_v, "A": dp_a, "P": dp_p}[eng].tile(
                    [128, ni, HW, HW], BF16, name="d" + eng, tag="D" + eng
                )
                if eng == "V":
                    nc.vector.tensor_scalar(
                        out=d_t, in0=win,
                        scalar1=wt[:, u : u + 1], scalar2=None, op0=ALU.min,
                    )
                    s_mat = m2g[g]
                elif eng == "A":
                    nc.scalar.activation(
                        out=d_t, in_=win, func=ACTF.Abs,
                        bias=nwt[:, u : u + 1], scale=1.0,
                    )
                    s_mat = gsel[g]
                else:
                    nc.gpsimd.tensor_scalar(
                        out=d_t, in0=win,
                        scalar1=wt[:, u : u + 1], scalar2=None, op0=ALU.min,
                    )
                    s_mat = m2g[g]
                tf = d_t.rearrange("p a b c -> p (a b c)")
                for i in range(ni):
                    img = i0 + i
                    ub, ui = u_ps_at(img)
                    for ck in range(NCK):
                        off = i * S + ck * CK
                        nc.tensor.matmul(
                            ub[0:CK, ui, ck, :],
                            tf[:, off : off + CK],
                            s_mat,
                            start=False, stop=False, skip_group_check=True,
                            tile_position=(0, 0),
                        )


            def emit_evac_stats(img, last):
                for ck in range(NCK):
                    uc = u_pair[img // 2][0:CK, img % 2, ck, :]
                    nc.tensor.matmul(
                        s2_ps, uc, uc,
                        start=False, stop=False, skip_group_check=True,
                        tile_position=(0, 0),
                    )
                    nc.tensor.matmul(
                        s1_ps, uc, ones_col,
                        start=False, stop=False, skip_group_check=True,
                        tile_position=(0, 0),
                    )

            for eng, g, j, i0, i1 in ops_a:
                emit_unit(eng, g, j, i0, i1)
            # S_x lands in psum via free identity matmuls (f32 moving, 8 cols)
            for img in range(N_IMG):
                ub, ui = u_ps_at(img)
                for ck in range(NCK):
                    nc.tensor.matmul(
                        ub[0:CK, ui, ck, :], idm, sx[0:CK, img, ck, :],
                        start=False, stop=False, skip_group_check=True,
                        tile_position=(0, 0),
                    )
            for eng, g, j, i0, i1 in ops_b:
                emit_unit(eng, g, j, i0, i1)
            for pair in range(N_IMG // 2):
                i0 = 2 * pair
                ub, ui = u_ps_at(i0)
                dst = u_pair[pair]
                if pair % 2 == 0 or pair == 3:
                    nc.vector.tensor_copy(
                        out=dst, in_=ub[0:CK, ui : ui + 2, :, :]
                    )
                else:
                    nc.scalar.copy(out=dst, in_=ub[0:CK, ui : ui + 2, :, :])
                emit_evac_stats(i0, last=False)
                emit_evac_stats(i0 + 1, last=i0 + 1 == N_IMG - 1)

            # --- BN chain --------------------------------------------------
            mv = small.tile([O_PER_CORE, 9], F32)
            nc.vector.tensor_scalar(
                out=mv, in0=s_ps_raw[0:O_PER_CORE, 0:9],
                scalar1=1.0 / NTOT, scalar2=None, op0=ALU.mult,
            )
            dg = small.tile([O_PER_CORE, 8], F32)
            nc.vector.tensor_tensor(out=dg, in0=mv[:, 0:8], in1=i8, op=ALU.mult)
            eu2 = small.tile([O_PER_CORE, 1], F32)
            nc.vector.tensor_reduce(
                out=eu2, in_=dg, op=ALU.add, axis=mybir.AxisListType.X
            )
            nvar = small.tile([O_PER_CORE, 1], F32)
            nc.vector.scalar_tensor_tensor(
                out=nvar, in0=mv[:, 8:9], scalar=mv[:, 8:9], in1=eu2,
                op0=ALU.mult, op1=ALU.subtract,
            )
            stdv = small.tile([O_PER_CORE, 1], F32)
            nc.scalar.activation(
                out=stdv, in_=nvar, func=ACTF.Sqrt, bias=eps_sb, scale=-1.0
            )
            rstd = small.tile([O_PER_CORE, 1], F32)
            nc.vector.reciprocal(out=rstd, in_=stdv)
            # A = -gamma * rstd ; B = beta - A * mean
            nc.vector.tensor_tensor(
                out=ab_sb[:, 0:1], in0=rstd, in1=ngam, op=ALU.mult
            )
            # B = beta - A*mean in one fused op: (beta * 1) - (A*mean)
            t2 = small.tile([O_PER_CORE, 1], F32)
            nc.vector.tensor_tensor(
                out=t2, in0=ab_sb[:, 0:1], in1=mv[:, 8:9], op=ALU.mult
            )
            nc.vector.scalar_tensor_tensor(
                out=ab_sb[:, 1:2], in0=beta, scalar=1.0, in1=t2,
                op0=ALU.mult, op1=ALU.subtract,
            )
            # diag(A)|diag(B) [8, 16] in one DVE op pair, then broadcast to
            # [112, 7, 8] by K=8 all-ones matmuls (sum over the diag rows)
            dab = small.tile([O_PER_CORE, 2, 8], F32)
            for t in range(2):
                nc.vector.tensor_scalar(
                    out=dab[:, t, :], in0=i8,
                    scalar1=ab_sb[:, t : t + 1], scalar2=None, op0=ALU.mult,
                )
                for ck in range(NCK):
                    nc.tensor.matmul(
                        abc_ps[0:CK, t, ck, :],
                        ones8r,
                        dab[:, t, :],
                        start=False, stop=False, skip_group_check=True,
                        tile_position=(0, 0),
                    )
            nc.vector.tensor_copy(out=ab_bc, in_=abc_ps)

            # --- affine + output ------------------------------------------
            AFF = {0: "P", 1: "P", 2: "P", 3: "P", 4: "P", 5: "P", 6: "P", 7: "P"}
            for img in (0, 1, 2, 3, 4, 5, 6, 7):
                tmp = t_pool.tile([CK, NCK, O_PER_CORE], F32, name="t", tag="T")
                tt = {"V": nc.vector, "P": nc.gpsimd}[AFF[img]]
                tt.tensor_tensor(
                    out=tmp, in0=u_pair[img // 2][0:CK, img % 2, :, :],
                    in1=ab_bc[0:CK, 0, :, :], op=ALU.mult,
                )
                tt.tensor_tensor(
                    out=y_sb[0:CK, img, :, :], in0=tmp, in1=ab_bc[0:CK, 1, :, :],
                    op=ALU.add,
                )
                if img == 3:
                    nc.scalar.dma_start(
                        out=y_out[:, 0:4, :, :], in_=y_sb[0:CK, 0:4, :, :]
                    )
                elif img == 7:
                    nc.sync.dma_start(
                        out=y_out[:, 4:8, :, :], in_=y_sb[0:CK, 4:8, :, :]
                    )
    nc.finalize()
    return nc


_NC_CACHE: dict = {}


def _get_nc() -> bass.Bass:
    if "nc" not in _NC_CACHE:
        _NC_CACHE["nc"] = _build_nc()
    return _NC_CACHE["nc"]


def _bf16(a):
    import ml_dtypes

    return np.ascontiguousarray(a).astype(ml_dtypes.bfloat16)


def _prep_x(x):
    """[8, 32, 28, 28] f32 -> (xx bf16 [128, 8*960], sx f32 [112, 8, 7, 8]).

    xx: zero-pad each image to 30x32 (row stride 32), bf16, replicate the 32
    channels into 4 partition slots, images contiguous along the free dim.
    sx[p, img, ck, o] = sum_{c, j in minset(g(o), img)} x_c(s + d_j) with
    s = ck*112 + p, from the bf16-rounded x (matches device min inputs).
    """
    xp = np.zeros((N_IMG, C_IN, HP, WP), dtype=np.float32)
    xp[:, :, 1 : 1 + HW, 1 : 1 + HW] = x
    xb16 = _bf16(xp)
    xb = xb16.reshape(N_IMG, C_IN, PADN)
    xx = np.ascontiguousarray(
        np.tile(xb, (1, 4, 1)).transpose(1, 0, 2).reshape(128, N_IMG * PADN)
    )

    csum = xb16.astype(np.float32).sum(axis=1)  # [8, 30, 32]
    wins = {}
    for j in range(9):
        dy, dx = divmod(j, 3)
        wins[j] = csum[:, dy : dy + HW, dx : dx + HW].reshape(N_IMG, S)
    sx = np.zeros((CK, N_IMG, NCK, O_PER_CORE), dtype=np.float32)
    for g in range(N_GRP):
        for img in range(N_IMG):
            taps = _min_taps(g, img)
            tot = np.zeros(S, dtype=np.float32)
            for j in taps:
                tot += wins[j][img]
            sxi = tot.reshape(NCK, CK).T  # [112, 7]
            for o in range(4 * g, 4 * g + 4):
                sx[:, img, :, o] = sxi
    return xx, sx


def _in_maps(x, W, gamma, beta):
    x = np.ascontiguousarray(x, dtype=np.float32)
    W = np.asarray(W, dtype=np.float32)
    gamma = np.asarray(gamma, dtype=np.float32)
    beta = np.asarray(beta, dtype=np.float32)
    xx, sx = _prep_x(x)

    slot = np.arange(128) // 32
    gmat = (slot[:, None] == np.arange(4)[None, :]).astype(np.float32)
    pb = np.zeros((128, PB_COLS), dtype=np.float32)
    for g in range(N_GRP):
        pb[:, PB_M2G + 8 * g + 4 * g : PB_M2G + 8 * g + 4 * g + 4] = -2.0 * gmat
        pb[:, PB_G + 8 * g + 4 * g : PB_G + 8 * g + 4 * g + 4] = gmat
    pb = _bf16(pb)

    # The dropped sum_{c,j in minset} w must be image-INDEPENDENT per channel
    # for BN to absorb it. Where the minset varies by image (per-image engine
    # splits), compensate the delta vs the img-0 set inside sx (per core).
    base_set = {g: _min_taps(g, 0) for g in range(N_GRP)}
    maps = []
    for core in range(N_CORES):
        base = core * O_PER_CORE
        sxc = sx.copy()
        for o in range(O_PER_CORE):
            g = o // 4
            wjsum = W[base + o].reshape(C_IN, 9).sum(axis=0)  # [9]
            for img in range(N_IMG):
                cur = _min_taps(g, img)
                delta = sum(wjsum[j] for j in cur - base_set[g]) - sum(
                    wjsum[j] for j in base_set[g] - cur
                )
                if delta:
                    sxc[:, img, :, o] += np.float32(delta)
        # partition p = slot*32 + c serves channel 4g+slot for unit (g,j)
        w8 = W[base : base + O_PER_CORE].reshape(N_GRP, 4, C_IN, 9)
        wt = w8.transpose(1, 2, 0, 3).reshape(128, N_GRP * 9)
        pf = np.zeros((128, PF_COLS), dtype=np.float32)
        pf[:, PF_WT : PF_WT + 18] = wt
        pf[:, PF_NWT : PF_NWT + 18] = -wt
        pf[0:O_PER_CORE, PF_NGAM] = -gamma[base : base + O_PER_CORE]
        pf[0:O_PER_CORE, PF_BETA] = beta[base : base + O_PER_CORE]
        pf[0:O_PER_CORE, PF_EPS] = EPS
        pf[:, PF_ONE] = 1.0
        pf[0:O_PER_CORE, PF_ONEROW : PF_ONEROW + CK] = 1.0
        pf[0:O_PER_CORE, PF_I8 : PF_I8 + 8] = np.eye(O_PER_CORE, dtype=np.float32)
        maps.append(
            {
                "xx": xx,
                "sx": sxc,
                "idm": np.eye(CK, dtype=np.float32),
                "pf": pf,
                "pb": pb,
            }
        )
    return maps


def _gather(results) -> np.ndarray:
    y = np.empty((N_IMG, O_TOT, S), dtype=np.float32)
    for core in range(N_CORES):
        yo = np.asarray(results[core]["y"], dtype=np.float32)  # [112, 8, 7, 8]
        # y[img, base+o, ck*112 + p] = yo[p, img, ck, o]
        yc = yo.transpose(1, 3, 2, 0).reshape(N_IMG, O_PER_CORE, S)
        y[:, core * O_PER_CORE : (core + 1) * O_PER_CORE, :] = yc
    return y.reshape(N_IMG, O_TOT, HW, HW)


def run(x, W, gamma, beta, trace=False, **trace_kwargs):
    nc = _get_nc()
    maps = _in_maps(x, W, gamma, beta)
    res = run_bass_kernel_spmd(
        nc, maps, list(range(N_CORES)), trace=trace, **trace_kwargs
    )
    return _gather(res.results), res


def kernel(x, W, gamma, beta) -> np.ndarray:
    y, _ = run(x, W, gamma, beta)
    return y
